# revision 1
# baseline (speedup 1.0000x reference)
"""DeepFM-style embedding reduction kernel for 8 Trainium2 NeuronCores.

Model (reference):
    embf    = emb^T @ x                  # (E,)  E=16, F=2M
    squ_sum = (emb*emb)^T @ (x*x)        # (E,)
    fm      = 0.5 * (embf^2 - squ_sum)
    h       = relu(relu(embf@w1.T+b1)@w2.T+b2)
    out     = sigmoid(concat(h, fm, embf@w_log.T+b_log) @ w_out.T + b_out)

The F=2M reduction is memory bound (emb is 128MB).  Sharding: rows (feature
dim) split across 8 cores.  Each core computes partial embf / squ_sum via:
  - DVE:  scaled = emb * broadcast(x)        (elementwise, fp32)
  - ACT:  scaled = scaled * scaled (in-place square, after first col-sum)
  - PE :  ones-matmul column sums accumulated in PSUM
Host gathers the 8 partial (16+16)-vectors, sums, and applies the tiny MLP.
Note squ_sum = sum((x*emb)^2), so no separate x^2/emb^2 passes are needed.
"""

import numpy as np

F = 2_000_000
E = 16
P = 128
NCORES = 8
CT = 1954            # free-dim columns per partition per core
S = P * CT           # 250112 rows per core shard (7*S + 249216 = F + 896 pad)
W = 448              # column-tile width (per tile: [128, W*16] fp32 = 3.7MB)

_cache = {}


def _build_nc(ct, w):
    from contextlib import ExitStack

    import concourse.bacc as bacc
    import concourse.bass as bass
    import concourse.tile as tile
    from concourse import mybir

    f32 = mybir.dt.float32
    f32r = mybir.dt.float32r
    nc = bacc.Bacc("TRN2", debug=False, num_devices=NCORES)
    x_d = nc.dram_tensor("xs", [P, ct], f32, kind="ExternalInput").ap()
    emb_d = nc.dram_tensor("embs", [P, ct * E], f32, kind="ExternalInput").ap()
    out_d = nc.dram_tensor("out", [1, 2 * E], f32, kind="ExternalOutput").ap()

    tiles = [(c0, min(w, ct - c0)) for c0 in range(0, ct, w)]
    # Split the embf column-sums between PE (fp32 matmuls, 4-pass) and DVE
    # (strided reduce) to balance engine load. s_pe[t] = #512-col matmul
    # slices handled by the PE for tile t; the tail goes to the DVE.
    s_pe = [(wt * E // 512) * 3 // 7 for _, wt in tiles]
    nmm_s = sum(s_pe)
    # ACT squares in chunks so sq tiles stay small; chunk = SQCH columns
    SQCH = 1792
    def q_slices(ncol):
        out = []
        for k0 in range(0, ncol, SQCH):
            cw = min(SQCH, ncol - k0)
            for j0 in range(0, cw, 512):
                out.append((k0, j0, min(512, cw - j0)))
        return out
    nmm = sum(len(q_slices(wt * E)) for _, wt in tiles)

    with ExitStack() as ctx:
        tc = ctx.enter_context(tile.TileContext(nc))
        embp = ctx.enter_context(tc.tile_pool(name="embp", bufs=3))
        sclp = ctx.enter_context(tc.tile_pool(name="sclp", bufs=3))
        sqp = ctx.enter_context(tc.tile_pool(name="sqp", bufs=2))
        xp = ctx.enter_context(tc.tile_pool(name="xp", bufs=3))
        singles = ctx.enter_context(tc.tile_pool(name="singles", bufs=1))
        psum = ctx.enter_context(tc.tile_pool(name="psum", bufs=1, space="PSUM"))

        ones = singles.tile([P, 1], f32)
        nc.vector.memset(ones, 1.0)

        ps_s = psum.tile([1, 512], f32, tag="ps_s")
        ps_q = psum.tile([1, 512], f32, tag="ps_q")
        ps_f = psum.tile([1, E], f32, tag="ps_f")
        # per-(tile, e) fp32 partial sums from the DVE-reduced columns
        acc_all = singles.tile([P, len(tiles) * E], f32)

        i_s = 0
        i_q = 0
        for t, (c0, wt) in enumerate(tiles):
            embt = embp.tile([P, wt * E], f32, tag="embt")
            nc.sync.dma_start(out=embt[:], in_=emb_d[:, c0 * E:(c0 + wt) * E])
            xt = xp.tile([P, wt], f32, tag="xt")
            nc.sync.dma_start(out=xt[:], in_=x_d[:, c0:c0 + wt])

            scaled = sclp.tile([P, wt * E], f32, tag="scaled")
            nc.vector.tensor_mul(
                out=scaled[:].rearrange("p (c e) -> p c e", e=E),
                in0=embt[:].rearrange("p (c e) -> p c e", e=E),
                in1=xt[:].unsqueeze(2).broadcast_to([P, wt, E]),
            )

            ncol = wt * E
            # embf column sums, part 1: fp32 matmuls on the first s_pe[t]
            # 512-col slices (psum col n accumulates (c%32, e) = (n>>4, n&15))
            pe_cols = s_pe[t] * 512
            for j0 in range(0, pe_cols, 512):
                nc.tensor.matmul(
                    ps_s[0:1, 0:512],
                    ones[:, 0:1],
                    scaled[:, j0:j0 + 512],
                    start=(i_s == 0),
                    stop=(i_s == nmm_s - 1),
                    skip_group_check=True,
                )
                i_s += 1
            # part 2: DVE strided reduce over the remaining columns (fp32)
            if ncol > pe_cols:
                nc.vector.reduce_sum(
                    out=acc_all[:, t * E:(t + 1) * E],
                    in_=scaled[:, pe_cols:ncol].rearrange("p (c e) -> p e c",
                                                          e=E),
                    axis=mybir.AxisListType.X,
                )
            else:
                nc.vector.memset(acc_all[:, t * E:(t + 1) * E], 0.0)

            # square on the scalar engine (ACT) in chunks, then fp32r
            # col-sums on the PE (squ is a positive sum; fp32r rounding is
            # negligible). Chunk starts are 16-aligned so psum col n keeps
            # e = n % 16.
            for k0 in range(0, ncol, SQCH):
                cw = min(SQCH, ncol - k0)
                sq = sqp.tile([P, SQCH], f32, tag="sq")
                nc.scalar.square(out=sq[:, 0:cw].bitcast(f32r),
                                 in_=scaled[:, k0:k0 + cw])
                for j0 in range(0, cw, 512):
                    nn = min(512, cw - j0)
                    nc.tensor.matmul(
                        ps_q[0:1, 0:nn],
                        ones[:, 0:1].bitcast(f32r),
                        sq[:, j0:j0 + nn].bitcast(f32r),
                        start=(i_q == 0),
                        stop=(i_q == nmm - 1),
                        skip_group_check=True,
                    )
                    i_q += 1

        # embf: sum per-tile DVE accs, partition-reduce via one fp32 matmul,
        # then add the PE path (ps_s) reduced over its 32 c-groups
        acc_sum = singles.tile([P, E], f32)
        nc.vector.reduce_sum(
            out=acc_sum[:],
            in_=acc_all[:].rearrange("p (t e) -> p e t", e=E),
            axis=mybir.AxisListType.X,
        )
        nc.tensor.matmul(ps_f[0:1, :], ones[:, 0:1], acc_sum[:],
                         start=True, stop=True)

        out_sb = singles.tile([1, 2 * E], f32)
        if nmm_s > 0:
            tmp_s = singles.tile([1, E], f32)
            nc.vector.reduce_sum(
                out=tmp_s[0:1, :],
                in_=ps_s[:].rearrange("p (c e) -> p e c", e=E),
                axis=mybir.AxisListType.X,
            )
            nc.vector.tensor_add(out=out_sb[0:1, 0:E], in0=tmp_s[0:1, :],
                                 in1=ps_f[0:1, :])
        else:
            nc.vector.tensor_copy(out=out_sb[0:1, 0:E], in_=ps_f[0:1, :])
        nc.vector.reduce_sum(
            out=out_sb[0:1, E:2 * E],
            in_=ps_q[:].rearrange("p (c e) -> p e c", e=E),
            axis=mybir.AxisListType.X,
        )
        nc.sync.dma_start(out=out_d, in_=out_sb[:])

    nc.compile()
    return nc


def _shard_inputs(x, emb, ct):
    """Shard x (F,) and emb (F,E) row-wise into NCORES pieces of P*ct rows."""
    s = P * ct
    total = s * NCORES
    in_maps = []
    for k in range(NCORES):
        lo, hi = k * s, min((k + 1) * s, F)
        if hi - lo == s and lo < F:
            xs = np.ascontiguousarray(x[lo:hi]).reshape(P, ct)
            es = np.ascontiguousarray(emb[lo:hi]).reshape(P, ct * E)
        else:
            xs = np.zeros((s,), np.float32)
            es = np.zeros((s, E), np.float32)
            if lo < F:
                xs[: hi - lo] = x[lo:hi]
                es[: hi - lo] = emb[lo:hi]
            xs = xs.reshape(P, ct)
            es = es.reshape(P, ct * E)
        in_maps.append({"xs": xs, "embs": es})
    assert total >= F
    return in_maps


def _ensure_ntff_hook():
    """The agent image's antenv lacks axon_hooks; provide it + register the
    ctypes NTFF profiling hook against the axon PJRT .so (trace-only path)."""
    import sys
    import types

    try:
        from antenv.axon_hooks import get_axon_ntff_profile_hook  # noqa: F401
        return
    except ImportError:
        pass
    mod = types.ModuleType("antenv.axon_hooks")
    _h = [None]
    mod.set_axon_ntff_profile_hook = lambda h: _h.__setitem__(0, h)
    mod.get_axon_ntff_profile_hook = lambda: _h[0]
    sys.modules["antenv.axon_hooks"] = mod
    try:
        import antenv
        antenv.axon_hooks = mod
    except ImportError:
        pass

    import contextlib
    import ctypes

    so_path = "/opt/axon/libaxon_pjrt.so"
    try:
        lib = ctypes.CDLL(so_path)
    except OSError:
        return
    if not hasattr(lib, "axon_start_nrt_profile"):
        return
    lib.axon_start_nrt_profile.argtypes = [ctypes.POINTER(ctypes.c_int64),
                                           ctypes.c_size_t]
    lib.axon_start_nrt_profile.restype = ctypes.c_int64
    lib.axon_stop_nrt_profile.argtypes = [ctypes.c_char_p]
    lib.axon_stop_nrt_profile.restype = ctypes.c_int64

    @contextlib.contextmanager
    def _hook(output_dir, device_ids):
        import jax
        jax.devices()
        if device_ids:
            ids = (ctypes.c_int64 * len(device_ids))(*device_ids)
            rc = lib.axon_start_nrt_profile(ids, len(device_ids))
        else:
            rc = lib.axon_start_nrt_profile(None, 0)
        if rc != 0:
            raise RuntimeError(f"axon_start_nrt_profile rc={rc}")
        try:
            yield
        finally:
            n = lib.axon_stop_nrt_profile(str(output_dir).encode())
            print(f"ntff profile: {n} file(s) -> {output_dir}")

    mod.set_axon_ntff_profile_hook(_hook)


def _run_device(x, emb, trace=False):
    from concourse.bass_utils import run_bass_kernel_spmd

    if trace:
        _ensure_ntff_hook()
    if "nc" not in _cache:
        _cache["nc"] = _build_nc(CT, W)
    nc = _cache["nc"]
    in_maps = _shard_inputs(x, emb, CT)
    res = run_bass_kernel_spmd(nc, in_maps, core_ids=list(range(NCORES)),
                               trace=trace)
    parts = np.stack([r["out"].reshape(2 * E) for r in res.results])  # [8, 32]
    totals = parts.sum(axis=0, dtype=np.float32)
    return totals[:E], totals[E:], res


def _mlp_head(embf, squ, w_log, b_log, w1, b1, w2, b2, w_out, b_out):
    embf = embf.astype(np.float32)
    squ = squ.astype(np.float32)
    logistic = embf @ w_log.T + b_log                       # (1,)
    fm = 0.5 * (embf * embf - squ)                          # (E,)
    h = np.maximum(embf @ w1.T + b1, 0.0)
    h = np.maximum(h @ w2.T + b2, 0.0)
    concat = np.concatenate([h, fm, logistic]).astype(np.float32)
    logit = concat @ w_out.T + b_out
    return (1.0 / (1.0 + np.exp(-logit))).astype(np.float32)


def kernel(x, emb, w_log, b_log, w1, b1, w2, b2, w_out, b_out, _trace=False):
    x = np.asarray(x, np.float32)
    emb = np.asarray(emb, np.float32)
    embf, squ, res = _run_device(x, emb, trace=_trace)
    out = _mlp_head(embf, squ,
                    np.asarray(w_log, np.float32), np.asarray(b_log, np.float32),
                    np.asarray(w1, np.float32), np.asarray(b1, np.float32),
                    np.asarray(w2, np.float32), np.asarray(b2, np.float32),
                    np.asarray(w_out, np.float32), np.asarray(b_out, np.float32))
    if _trace:
        kernel.last_results = res
    return out



# revision 3
# speedup vs baseline: 1.8521x; 1.8521x over previous
"""DeepFM-style embedding reduction kernel for 8 Trainium2 NeuronCores.

Model (reference):
    embf    = emb^T @ x                  # (E,)  E=16, F=2M
    squ     = (emb*emb)^T @ (x*x)        # (E,)
    fm      = 0.5 * (embf^2 - squ)
    h       = relu(relu(embf@w1.T+b1)@w2.T+b2)
    out     = sigmoid(concat(h, fm, embf@w_log.T+b_log) @ w_out.T + b_out)

The F=2M reduction is memory bound (emb is 128MB fp32).  The final output is
a sigmoid deep in its exponential tail (~1.8e-8), so output rel-err ~= abs
logit error; the fm term amplifies embf error by ~2*|embf|*w_out, which rules
out bf16/fp16 input streams (measured 20%/5% output rel err).  int16
fixed-point (scale folded on host) gives 15 mantissa bits -> ~0.3% output
rel err while still halving HBM traffic vs fp32.

Device-side per core (rows split across 8 cores, e-major layout [P, E, CT]):
  - DVE  tensor_tensor_reduce: scaled_bf16 = emb_i16 * x_i16 (fp32 internal),
         fused accum -> per-partition embf partial (fp32).  2x DVE mode.
  - ACT  activation(Square) with fused accum_out -> per-partition squ
         partial (fp32).  scaled values are integer-scaled products; bf16
         rounding of them is statistically harmless for the positive squ sum.
  - No PE, no DVE reductions (tensor_reduce is 1x-mode and slow).
  - DMA: x (0.5MB) on the ACT HWDGE ring; emb (8MB) as 1MB chunks on the
         sync ring (FIFO -> chunk k completes early, pipelining compute).
Host: int16 quantize + e-major reshape, final 128-partition + 8-core sum in
float64, tiny MLP head in numpy.
"""

import numpy as np

F = 2_000_000
E = 16
P = 128
NCORES = 8
CT = 1954            # free-dim columns per partition per core
S = P * CT           # 250112 rows per core shard (8*S = 2000896 >= F)
ROWS_PER_DMA = 2     # e-rows per emb DMA chunk -> 1MB transfers

_cache = {}


def _build_nc():
    from contextlib import ExitStack

    import concourse.bacc as bacc
    import concourse.tile as tile
    from concourse import mybir

    i16 = mybir.dt.int16
    bf16 = mybir.dt.bfloat16
    f32 = mybir.dt.float32
    nc = bacc.Bacc("TRN2", debug=False, num_devices=NCORES)
    x_d = nc.dram_tensor("xq", [P, CT], i16, kind="ExternalInput").ap()
    emb_d = nc.dram_tensor("embq", [P, E * CT], i16, kind="ExternalInput").ap()
    out_d = nc.dram_tensor("out", [P, 2 * E], f32, kind="ExternalOutput").ap()

    nchunk = E // ROWS_PER_DMA
    with ExitStack() as ctx:
        tc = ctx.enter_context(tile.TileContext(nc))
        singles = ctx.enter_context(tc.tile_pool(name="singles", bufs=1))
        sclp = ctx.enter_context(tc.tile_pool(name="sclp", bufs=3))
        sqp = ctx.enter_context(tc.tile_pool(name="sqp", bufs=3))

        x_sb = singles.tile([P, CT], i16, name="x_sb")
        nc.scalar.dma_start(out=x_sb[:], in_=x_d)
        emb_sb = []
        for k in range(nchunk):
            et = singles.tile([P, ROWS_PER_DMA * CT], i16, name=f"embc{k}")
            nc.sync.dma_start(
                out=et[:],
                in_=emb_d[:, k * ROWS_PER_DMA * CT:(k + 1) * ROWS_PER_DMA * CT],
            )
            emb_sb.append(et)

        pe_t = singles.tile([P, E], f32, name="pe_t")   # embf partials
        pq_t = singles.tile([P, E], f32, name="pq_t")   # squ partials

        for e in range(E):
            chunk, off = divmod(e, ROWS_PER_DMA)
            erow = emb_sb[chunk][:, off * CT:(off + 1) * CT]
            scaled = sclp.tile([P, CT], bf16, tag="scaled")
            nc.vector.scalar_tensor_tensor(
                out=scaled[:],
                in0=erow,
                scalar=1.0,
                in1=x_sb[:],
                op0=mybir.AluOpType.mult,
                op1=mybir.AluOpType.mult,
                accum_out=pe_t[:, e:e + 1],
            )
            sq = sqp.tile([P, CT], bf16, tag="sq")
            nc.scalar.activation(
                out=sq[:],
                in_=scaled[:],
                func=mybir.ActivationFunctionType.Square,
                accum_out=pq_t[:, e:e + 1],
            )

        nc.sync.dma_start(out=out_d[:, 0:E], in_=pe_t[:])
        nc.scalar.dma_start(out=out_d[:, E:2 * E], in_=pq_t[:])

    nc.compile()
    return nc


def _prep_inputs(x, emb):
    """int16-quantize x/emb (scales folded out) and shard e-major per core."""
    x = np.asarray(x, np.float32).reshape(F)
    emb = np.asarray(emb, np.float32).reshape(F, E)
    sx = float(np.max(np.abs(x))) / 32767.0
    se = float(np.max(np.abs(emb))) / 32767.0
    sx = sx if sx > 0 else 1.0
    se = se if se > 0 else 1.0
    xq = np.clip(np.rint(x * (1.0 / sx)), -32767, 32767).astype(np.int16)
    eq = np.clip(np.rint(emb * (1.0 / se)), -32767, 32767).astype(np.int16)
    total = NCORES * S
    if total > F:
        xq = np.concatenate([xq, np.zeros(total - F, np.int16)])
        eq = np.concatenate([eq, np.zeros((total - F, E), np.int16)])
    in_maps = []
    for k in range(NCORES):
        xs = xq[k * S:(k + 1) * S].reshape(P, CT)
        es = eq[k * S:(k + 1) * S].reshape(P, CT, E).transpose(0, 2, 1)
        in_maps.append({
            "xq": np.ascontiguousarray(xs),
            "embq": np.ascontiguousarray(es).reshape(P, E * CT),
        })
    return in_maps, sx, se


def _ensure_ntff_hook():
    """The agent image's antenv lacks axon_hooks; provide it + register the
    ctypes NTFF profiling hook against the axon PJRT .so (trace-only path)."""
    import sys
    import types

    try:
        from antenv.axon_hooks import get_axon_ntff_profile_hook  # noqa: F401
        return
    except ImportError:
        pass
    mod = types.ModuleType("antenv.axon_hooks")
    _h = [None]
    mod.set_axon_ntff_profile_hook = lambda h: _h.__setitem__(0, h)
    mod.get_axon_ntff_profile_hook = lambda: _h[0]
    sys.modules["antenv.axon_hooks"] = mod
    try:
        import antenv
        antenv.axon_hooks = mod
    except ImportError:
        pass

    import contextlib
    import ctypes

    so_path = "/opt/axon/libaxon_pjrt.so"
    try:
        lib = ctypes.CDLL(so_path)
    except OSError:
        return
    if not hasattr(lib, "axon_start_nrt_profile"):
        return
    lib.axon_start_nrt_profile.argtypes = [ctypes.POINTER(ctypes.c_int64),
                                           ctypes.c_size_t]
    lib.axon_start_nrt_profile.restype = ctypes.c_int64
    lib.axon_stop_nrt_profile.argtypes = [ctypes.c_char_p]
    lib.axon_stop_nrt_profile.restype = ctypes.c_int64

    @contextlib.contextmanager
    def _hook(output_dir, device_ids):
        import jax
        jax.devices()
        if device_ids:
            ids = (ctypes.c_int64 * len(device_ids))(*device_ids)
            rc = lib.axon_start_nrt_profile(ids, len(device_ids))
        else:
            rc = lib.axon_start_nrt_profile(None, 0)
        if rc != 0:
            raise RuntimeError(f"axon_start_nrt_profile rc={rc}")
        try:
            yield
        finally:
            n = lib.axon_stop_nrt_profile(str(output_dir).encode())
            print(f"ntff profile: {n} file(s) -> {output_dir}")

    mod.set_axon_ntff_profile_hook(_hook)


def _run_device(x, emb, trace=False):
    from concourse.bass_utils import run_bass_kernel_spmd

    if trace:
        _ensure_ntff_hook()
    if "nc" not in _cache:
        _cache["nc"] = _build_nc()
    nc = _cache["nc"]
    in_maps, sx, se = _prep_inputs(x, emb)
    res = run_bass_kernel_spmd(nc, in_maps, core_ids=list(range(NCORES)),
                               trace=trace)
    parts = np.stack([r["out"].astype(np.float64) for r in res.results])
    totals = parts.sum(axis=(0, 1))              # [2E]
    embf = (totals[:E] * (sx * se)).astype(np.float32)
    squ = (totals[E:] * (sx * se) ** 2).astype(np.float32)
    return embf, squ, res


def _mlp_head(embf, squ, w_log, b_log, w1, b1, w2, b2, w_out, b_out):
    embf = embf.astype(np.float32)
    squ = squ.astype(np.float32)
    logistic = embf @ w_log.T + b_log                       # (1,)
    fm = 0.5 * (embf * embf - squ)                          # (E,)
    h = np.maximum(embf @ w1.T + b1, 0.0)
    h = np.maximum(h @ w2.T + b2, 0.0)
    concat = np.concatenate([h, fm, logistic]).astype(np.float32)
    logit = concat @ w_out.T + b_out
    return (1.0 / (1.0 + np.exp(-logit))).astype(np.float32)


def kernel(x, emb, w_log, b_log, w1, b1, w2, b2, w_out, b_out, _trace=False):
    x = np.asarray(x, np.float32)
    emb = np.asarray(emb, np.float32)
    embf, squ, res = _run_device(x, emb, trace=_trace)
    out = _mlp_head(embf, squ,
                    np.asarray(w_log, np.float32), np.asarray(b_log, np.float32),
                    np.asarray(w1, np.float32), np.asarray(b1, np.float32),
                    np.asarray(w2, np.float32), np.asarray(b2, np.float32),
                    np.asarray(w_out, np.float32), np.asarray(b_out, np.float32))
    if _trace:
        kernel.last_results = res
    return out


# revision 6
# speedup vs baseline: 1.9946x; 1.0770x over previous
"""DeepFM-style embedding reduction kernel for 8 Trainium2 NeuronCores.

Model (reference):
    embf    = emb^T @ x                  # (E,)  E=16, F=2M
    squ     = (emb*emb)^T @ (x*x)        # (E,)
    fm      = 0.5 * (embf^2 - squ)
    h       = relu(relu(embf@w1.T+b1)@w2.T+b2)
    out     = sigmoid(concat(h, fm, embf@w_log.T+b_log) @ w_out.T + b_out)

The F=2M reduction is memory bound (emb is 128MB fp32).  The final output is
a sigmoid deep in its exponential tail (~1.8e-8), so output rel-err ~= abs
logit error; the fm term amplifies embf error by ~2*|embf|*w_out, which rules
out bf16/fp16 input streams (measured 20%/5% output rel err).  int16
fixed-point (scale folded on host) gives 15 mantissa bits -> ~0.3% output
rel err while still halving HBM traffic vs fp32.

Device-side per core (rows split across 8 cores, e-major layout [P, E, CT]):
  - DVE  tensor_tensor_reduce: scaled_bf16 = emb_i16 * x_i16 (fp32 internal),
         fused accum -> per-partition embf partial (fp32).  2x DVE mode.
  - ACT  activation(Square) with fused accum_out -> per-partition squ
         partial (fp32).  scaled values are integer-scaled products; bf16
         rounding of them is statistically harmless for the positive squ sum.
  - No PE, no DVE reductions (tensor_reduce is 1x-mode and slow).
  - DMA: x (0.5MB) on the ACT HWDGE ring; emb (8MB) as 1MB chunks on the
         sync ring (FIFO -> chunk k completes early, pipelining compute).
Host: int16 quantize + e-major reshape, final 128-partition + 8-core sum in
float64, tiny MLP head in numpy.
"""

import numpy as np

F = 2_000_000
E = 16
P = 128
NCORES = 8
CT = 1954            # free-dim columns per partition per core
S = P * CT           # 250112 rows per core shard (8*S = 2000896 >= F)
ROWS_PER_DMA = 1     # e-rows per emb DMA chunk -> 0.5MB transfers
EMB_BUFS = 4         # chunk-pool depth: DMA k starts when compute frees k-4

_cache = {}


def _build_nc():
    from contextlib import ExitStack

    import concourse.bacc as bacc
    import concourse.tile as tile
    from concourse import mybir

    i16 = mybir.dt.int16
    bf16 = mybir.dt.bfloat16
    f32 = mybir.dt.float32
    nc = bacc.Bacc("TRN2", debug=False, num_devices=NCORES)
    x_d = nc.dram_tensor("xq", [P, CT], i16, kind="ExternalInput").ap()
    emb_d = nc.dram_tensor("embq", [P, E * CT], i16, kind="ExternalInput").ap()
    out_d = nc.dram_tensor("out", [P, 2 * E], f32, kind="ExternalOutput").ap()

    with ExitStack() as ctx:
        tc = ctx.enter_context(tile.TileContext(nc))
        singles = ctx.enter_context(tc.tile_pool(name="singles", bufs=1))
        embp = ctx.enter_context(tc.tile_pool(name="embp", bufs=EMB_BUFS))
        sclp = ctx.enter_context(tc.tile_pool(name="sclp", bufs=4))
        sqp = ctx.enter_context(tc.tile_pool(name="sqp", bufs=4))

        x_sb = singles.tile([P, CT], i16, name="x_sb")
        nc.scalar.dma_start(out=x_sb[:], in_=x_d)

        pe_t = singles.tile([P, E], f32, name="pe_t")   # embf partials
        pq_t = singles.tile([P, E], f32, name="pq_t")   # squ partials

        for e in range(E):
            et = embp.tile([P, ROWS_PER_DMA * CT], i16, tag="embc")
            nc.sync.dma_start(
                out=et[:],
                in_=emb_d[:, e * ROWS_PER_DMA * CT:(e + 1) * ROWS_PER_DMA * CT],
            )
            erow = et[:, 0:CT]
            scaled = sclp.tile([P, CT], bf16, tag="scaled")
            nc.vector.scalar_tensor_tensor(
                out=scaled[:],
                in0=erow,
                scalar=1.0,
                in1=x_sb[:],
                op0=mybir.AluOpType.mult,
                op1=mybir.AluOpType.mult,
                accum_out=pe_t[:, e:e + 1],
            )
            sq = sqp.tile([P, CT], bf16, tag="sq")
            nc.scalar.activation(
                out=sq[:],
                in_=scaled[:],
                func=mybir.ActivationFunctionType.Square,
                accum_out=pq_t[:, e:e + 1],
            )

        nc.sync.dma_start(out=out_d[:, 0:E], in_=pe_t[:])
        nc.scalar.dma_start(out=out_d[:, E:2 * E], in_=pq_t[:])

    nc.compile()
    return nc


def _prep_inputs(x, emb):
    """int16-quantize x/emb (scales folded out) and shard e-major per core."""
    x = np.asarray(x, np.float32).reshape(F)
    emb = np.asarray(emb, np.float32).reshape(F, E)
    sx = float(np.max(np.abs(x))) / 32767.0
    se = float(np.max(np.abs(emb))) / 32767.0
    sx = sx if sx > 0 else 1.0
    se = se if se > 0 else 1.0
    xq = np.clip(np.rint(x * (1.0 / sx)), -32767, 32767).astype(np.int16)
    eq = np.clip(np.rint(emb * (1.0 / se)), -32767, 32767).astype(np.int16)
    total = NCORES * S
    if total > F:
        xq = np.concatenate([xq, np.zeros(total - F, np.int16)])
        eq = np.concatenate([eq, np.zeros((total - F, E), np.int16)])
    in_maps = []
    for k in range(NCORES):
        xs = xq[k * S:(k + 1) * S].reshape(P, CT)
        es = eq[k * S:(k + 1) * S].reshape(P, CT, E).transpose(0, 2, 1)
        in_maps.append({
            "xq": np.ascontiguousarray(xs),
            "embq": np.ascontiguousarray(es).reshape(P, E * CT),
        })
    return in_maps, sx, se


def _ensure_ntff_hook():
    """The agent image's antenv lacks axon_hooks; provide it + register the
    ctypes NTFF profiling hook against the axon PJRT .so (trace-only path)."""
    import sys
    import types

    try:
        from antenv.axon_hooks import get_axon_ntff_profile_hook  # noqa: F401
        return
    except ImportError:
        pass
    mod = types.ModuleType("antenv.axon_hooks")
    _h = [None]
    mod.set_axon_ntff_profile_hook = lambda h: _h.__setitem__(0, h)
    mod.get_axon_ntff_profile_hook = lambda: _h[0]
    sys.modules["antenv.axon_hooks"] = mod
    try:
        import antenv
        antenv.axon_hooks = mod
    except ImportError:
        pass

    import contextlib
    import ctypes

    so_path = "/opt/axon/libaxon_pjrt.so"
    try:
        lib = ctypes.CDLL(so_path)
    except OSError:
        return
    if not hasattr(lib, "axon_start_nrt_profile"):
        return
    lib.axon_start_nrt_profile.argtypes = [ctypes.POINTER(ctypes.c_int64),
                                           ctypes.c_size_t]
    lib.axon_start_nrt_profile.restype = ctypes.c_int64
    lib.axon_stop_nrt_profile.argtypes = [ctypes.c_char_p]
    lib.axon_stop_nrt_profile.restype = ctypes.c_int64

    @contextlib.contextmanager
    def _hook(output_dir, device_ids):
        import jax
        jax.devices()
        if device_ids:
            ids = (ctypes.c_int64 * len(device_ids))(*device_ids)
            rc = lib.axon_start_nrt_profile(ids, len(device_ids))
        else:
            rc = lib.axon_start_nrt_profile(None, 0)
        if rc != 0:
            raise RuntimeError(f"axon_start_nrt_profile rc={rc}")
        try:
            yield
        finally:
            n = lib.axon_stop_nrt_profile(str(output_dir).encode())
            print(f"ntff profile: {n} file(s) -> {output_dir}")

    mod.set_axon_ntff_profile_hook(_hook)


def _run_device(x, emb, trace=False):
    from concourse.bass_utils import run_bass_kernel_spmd

    if trace:
        _ensure_ntff_hook()
    if "nc" not in _cache:
        _cache["nc"] = _build_nc()
    nc = _cache["nc"]
    in_maps, sx, se = _prep_inputs(x, emb)
    res = run_bass_kernel_spmd(nc, in_maps, core_ids=list(range(NCORES)),
                               trace=trace)
    parts = np.stack([r["out"].astype(np.float64) for r in res.results])
    totals = parts.sum(axis=(0, 1))              # [2E]
    embf = (totals[:E] * (sx * se)).astype(np.float32)
    squ = (totals[E:] * (sx * se) ** 2).astype(np.float32)
    return embf, squ, res


def _mlp_head(embf, squ, w_log, b_log, w1, b1, w2, b2, w_out, b_out):
    embf = embf.astype(np.float32)
    squ = squ.astype(np.float32)
    logistic = embf @ w_log.T + b_log                       # (1,)
    fm = 0.5 * (embf * embf - squ)                          # (E,)
    h = np.maximum(embf @ w1.T + b1, 0.0)
    h = np.maximum(h @ w2.T + b2, 0.0)
    concat = np.concatenate([h, fm, logistic]).astype(np.float32)
    logit = concat @ w_out.T + b_out
    return (1.0 / (1.0 + np.exp(-logit))).astype(np.float32)


def kernel(x, emb, w_log, b_log, w1, b1, w2, b2, w_out, b_out, _trace=False):
    x = np.asarray(x, np.float32)
    emb = np.asarray(emb, np.float32)
    embf, squ, res = _run_device(x, emb, trace=_trace)
    out = _mlp_head(embf, squ,
                    np.asarray(w_log, np.float32), np.asarray(b_log, np.float32),
                    np.asarray(w1, np.float32), np.asarray(b1, np.float32),
                    np.asarray(w2, np.float32), np.asarray(b2, np.float32),
                    np.asarray(w_out, np.float32), np.asarray(b_out, np.float32))
    if _trace:
        kernel.last_results = res
    return out
